# revision 19
# baseline (speedup 1.0000x reference)
"""Adaptive token pruner (entropy-gated cascaded db4 DWT) on 8 TRN2 NeuronCores.

Strategy (pure data parallel, 32 samples/core):
  - Each core receives its 32-sample shard of x as a flat row tensor
    (32*197 rows + 60 zero pad rows so every K-chunk DMA is 128 rows).
  - Each core receives the FULL cls_attention_map (rotated so its own 32
    samples are rows 0..31) and computes all 256 entropies locally; batch
    quantile thresholds reduce to rank comparisons (q25/q50 interpolation
    lies strictly between order stats 63/64 and 127/128), so
    level masks are m1 = rank>=128, m12 = rank>=64 with
    rank[b] = #{j : s[j] > s[b]}, s[b] = sum_n a*ln(a+1e-9) (= -ln2 * ent).
  - The 1/2/3-level lowpass DWT cascade along the 196 patch tokens is a
    banded matmul with seq as the contraction dim: y_sel = M_b^T @ patch,
    where M_b = D0 + m12*D1 + m1*D2 blends precomputed composite filter
    matrices (D0=C3p, D1=C2p-C3p, D2=W1-C2p); zero padding of shorter
    levels falls out exactly (blended columns are exact zeros).
  - Per sample: 4 fp32r matmuls (2 K-chunks x 2 PSUM banks), one
    PSUM->SBUF copy, grouped DMAs. CLS token rows are copied DRAM->DRAM.
"""

import numpy as np

import concourse.bass as bass
import concourse.mybir as mybir
import concourse.tile as tile
from concourse import bacc
from concourse.bass_utils import run_bass_kernel_spmd
from concourse.masks import make_identity

F32 = mybir.dt.float32
F32R = mybir.dt.float32r
AF = mybir.ActivationFunctionType
ALU = mybir.AluOpType

B, N, D = 256, 197, 768
NCORES = 8
BPC = B // NCORES          # 32 samples per core
NPAT = N - 1               # 196 patch tokens
L1, L2, L3 = 101, 54, 30   # DWT output lengths
TOUT = 1 + L1              # 102 output rows per sample
XROWS = BPC * N + 60       # flat x rows per core + pad so row chunks reach 256
G = 4                      # samples per DMA group

DB4_DEC_LO = np.array([
    -0.010597401784997278, 0.032883011666982945,
    0.030841381835986965, -0.18703481171888114,
    -0.02798376941698385, 0.6308807679295904,
    0.7148465705525415, 0.23037781330885523], dtype=np.float64)


def _build_w(n_in):
    """Banded matrix W (n_in, out_len) with y = W.T @ x equal to one level of
    zero-mode stride-2 db4 lowpass DWT (pytorch_wavelets conv semantics)."""
    h = DB4_DEC_LO[::-1]
    L = h.shape[0]
    out_len = (n_in + L - 1) // 2
    p = 2 * (out_len - 1) - n_in + L
    half = p // 2
    W = np.zeros((n_in, out_len), dtype=np.float64)
    for t in range(out_len):
        for l in range(L):
            n2 = 2 * t + l - half
            if 0 <= n2 < n_in:
                W[n2, t] += h[l]
    return W


def _build_dmat():
    """(128, 3, 2, 101) f32: D[p, m, j, t] = Dm[j*128+p, t], K padded 196->256
    with zeros (pad rows multiply don't-care x rows)."""
    W1 = _build_w(NPAT)              # (196, 101)
    W2 = _build_w(L1)                # (101, 54)
    W3 = _build_w(L2)                # (54, 30)
    C2 = W1 @ W2
    C3 = C2 @ W3
    C2p = np.zeros((NPAT, L1)); C2p[:, :L2] = C2
    C3p = np.zeros((NPAT, L1)); C3p[:, :L3] = C3
    Ds = np.stack([C3p, C2p - C3p, W1 - C2p])      # (3, 196, 101)
    Dk = np.zeros((3, 2 * 128, L1))
    Dk[:, :NPAT, :] = Ds
    return np.ascontiguousarray(
        Dk.reshape(3, 2, 128, L1).transpose(2, 0, 1, 3)).astype(np.float32)


DMAT = _build_dmat()

_NC_CACHE = {}


def _strided(ap, offset, dims):
    c = ap.copy()
    c.ap = c.ap[:0] + [list(d) for d in dims]
    c.offset = offset
    return c


def _build_nc():
    nc = bacc.Bacc("TRN2", target_bir_lowering=False, debug=False,
                   num_devices=NCORES)
    x = nc.dram_tensor("x", [XROWS, D], F32, kind="ExternalInput")
    attn = nc.dram_tensor("attn", [B, NPAT], F32, kind="ExternalInput")
    dmat = nc.dram_tensor("dmat", [128, 3, 2, L1], F32, kind="ExternalInput")
    out = nc.dram_tensor("out", [BPC * TOUT, D], F32, kind="ExternalOutput")
    levels_out = nc.dram_tensor("levels", [1, BPC], F32, kind="ExternalOutput")

    # Constants initialized in a raw-bass preamble (barrier'd) so Tile ops
    # reading them carry no semaphore waits.
    ident_t = nc.alloc_sbuf_tensor("ident_c", [128, 128], F32)
    biast_t = nc.alloc_sbuf_tensor("biast_c", [128, 1], F32)
    ones_t = nc.alloc_sbuf_tensor("ones_c", [1, 128], F32)
    make_identity(nc, ident_t.ap())
    nc.gpsimd.memset(biast_t.ap(), 1e-9)
    nc.gpsimd.memset(ones_t.ap(), 1.0)
    nc.all_engine_barrier()
    ident = ident_t.ap()
    biast = biast_t.ap()
    ones_row = ones_t.ap()

    from contextlib import ExitStack
    with tile.TileContext(nc) as tc, ExitStack() as ctx:
        const = ctx.enter_context(tc.tile_pool(name="const", bufs=1))
        ent = ctx.enter_context(tc.tile_pool(name="ent", bufs=1))
        pps = ctx.enter_context(tc.tile_pool(name="pps", bufs=1, space="PSUM"))
        xpool = ctx.enter_context(tc.tile_pool(name="xin", bufs=3))
        mbpool = ctx.enter_context(tc.tile_pool(name="mb", bufs=8))
        tmpool = ctx.enter_context(tc.tile_pool(name="mbtmp", bufs=2))
        stg = ctx.enter_context(tc.tile_pool(name="stg", bufs=3))
        ppool = ctx.enter_context(tc.tile_pool(name="psum", bufs=3, space="PSUM"))

        # ---- constants ----
        dm = const.tile([128, 3, 2, L1], F32)
        nc.sync.dma_start(dm[:], dmat.ap())

        # ---- entropy -> rank -> level masks prologue ----
        at = ent.tile([128, 2, NPAT], F32)
        nc.sync.dma_start(at[:], attn.ap().rearrange("(j p) d -> p j d", p=128))
        lg = ent.tile([128, 2, NPAT], F32)
        junk0 = ent.tile([128, NPAT], F32)
        junk1 = ent.tile([128, NPAT], F32)
        s_col = ent.tile([128, 2], F32)
        # absorber: pull the attn-DMA wait onto the DVE engine clock so the
        # table-lowered STT ops below carry at most one embedded wait
        absorb = ent.tile([128, 1], F32)
        nc.vector.tensor_copy(absorb[:], at[:, 0, 0:1])
        for j, junk in ((0, junk0), (1, junk1)):
            nc.scalar.activation(lg[:, j, :], at[:, j, :], AF.Ln,
                                 bias=biast[:])
            nc.vector.scalar_tensor_tensor(
                out=junk[:], in0=lg[:, j, :], scalar=1.0, in1=at[:, j, :],
                op0=ALU.mult, op1=ALU.mult, accum_out=s_col[:, j:j + 1])
        # s values of all 256 samples as one broadcast row
        s_row = ent.tile([1, B], F32)
        for j in range(2):
            tp = pps.tile([1, 128], F32, tag="tp")
            nc.tensor.transpose(tp[:], s_col[:, j:j + 1], ident[:])
            nc.vector.tensor_copy(s_row[:, j * 128:(j + 1) * 128], tp[:])
        # broadcast s_row to all 128 partitions via ones (K=1) matmul
        s_bc = ent.tile([128, B], F32)
        bc_ps = pps.tile([128, B], F32, tag="bc")
        nc.tensor.matmul(bc_ps[:], ones_row[:], s_row[:], start=True, stop=True)
        nc.vector.tensor_copy(s_bc[:], bc_ps[:])
        # rank for tile-0 samples (rows 0..31 are this core's own samples)
        cmp = ent.tile([128, B], F32)
        rank = ent.tile([128, 1], F32)
        nc.vector.tensor_scalar(
            out=cmp[:], in0=s_bc[:], scalar1=s_col[:, 0:1], scalar2=None,
            op0=ALU.is_gt, op1=ALU.add, accum_out=rank[:])
        mtile = ent.tile([128, 3], F32)   # cols: m1, m12, level
        nc.vector.tensor_scalar(
            out=mtile[:, 0:1], in0=rank[:], scalar1=128.0, scalar2=None,
            op0=ALU.is_ge)
        nc.vector.tensor_scalar(
            out=mtile[:, 1:2], in0=rank[:], scalar1=64.0, scalar2=None,
            op0=ALU.is_ge)
        # level = 3 - m1 - m12
        nc.vector.scalar_tensor_tensor(
            out=mtile[:, 2:3], in0=mtile[:, 0:1], scalar=-1.0,
            in1=mtile[:, 1:2], op0=ALU.mult, op1=ALU.subtract)
        nc.vector.tensor_scalar(
            out=mtile[:, 2:3], in0=mtile[:, 2:3], scalar1=3.0, scalar2=None,
            op0=ALU.add)
        mrow = ent.tile([1, 2 * BPC], F32)
        lev_row = ent.tile([1, BPC], F32)
        for col, dst in ((0, mrow[:, 0:BPC]), (1, mrow[:, BPC:2 * BPC]),
                         (2, lev_row[:])):
            tpm = pps.tile([1, 128], F32, tag="tp")
            nc.tensor.transpose(tpm[:], mtile[:, col:col + 1], ident[:])
            nc.vector.tensor_copy(dst, tpm[0:1, 0:BPC])
        nc.sync.dma_start(levels_out.ap(), lev_row[:])
        m_bc = ent.tile([128, 2 * BPC], F32)   # [:, i]=m1_i, [:, 32+i]=m12_i
        mb_ps = pps.tile([128, 2 * BPC], F32, tag="bc")
        nc.tensor.matmul(mb_ps[:], ones_row[:], mrow[:], start=True, stop=True)
        nc.vector.tensor_copy(m_bc[:], mb_ps[:])

        # ---- main loop: blended banded matmul per sample ----
        for g in range(BPC // G):
            b0 = g * G
            xt = xpool.tile([128, G, 2, D], F32R)
            for j in range(2):
                nc.gpsimd.dma_start(
                    xt[:, :, j, :],
                    _strided(x.ap(), (N * b0 + 1 + 128 * j) * D,
                             [[D, 128], [N * D, G], [1, D]]))
            st = stg.tile([L1, G, D], F32)
            for s in range(G):
                i = b0 + s
                mb = mbpool.tile([128, 2, L1], F32R)
                for j in range(2):
                    tmpb = tmpool.tile([128, L1], F32, tag="tmpb")
                    nc.vector.scalar_tensor_tensor(
                        out=tmpb[:], in0=dm[:, 1, j, :],
                        scalar=m_bc[:, BPC + i:BPC + i + 1],
                        in1=dm[:, 0, j, :], op0=ALU.mult, op1=ALU.add)
                    nc.vector.scalar_tensor_tensor(
                        out=mb[:, j, :], in0=dm[:, 2, j, :],
                        scalar=m_bc[:, i:i + 1],
                        in1=tmpb[:], op0=ALU.mult, op1=ALU.add)
                ps = ppool.tile([L1, D], F32)
                for j in range(2):
                    for n0, n1 in ((0, 512), (512, D)):
                        nc.tensor.matmul(
                            ps[:, n0:n1],
                            mb[:, j, :],
                            xt[:, s, j, n0:n1],
                            start=(j == 0), stop=(j == 1))
                if s % 2 == 0:
                    nc.vector.tensor_copy(st[:, s, :], ps[:])
                else:
                    nc.scalar.copy(st[:, s, :], ps[:])
            nc.sync.dma_start(
                _strided(out.ap(), (TOUT * b0 + 1) * D,
                         [[D, L1], [TOUT * D, G], [1, D]]),
                st[:])
        # CLS token rows: DRAM -> DRAM strided copy for all 32 samples
        nc.sync.dma_start(
            _strided(out.ap(), 0, [[TOUT * D, BPC], [1, D]]),
            _strided(x.ap(), 0, [[N * D, BPC], [1, D]]))
    nc.compile()
    return nc


def _get_nc():
    if "nc" not in _NC_CACHE:
        _NC_CACHE["nc"] = _build_nc()
    return _NC_CACHE["nc"]


def kernel(x, cls_attention_map, _bench_out=None):
    x = np.ascontiguousarray(np.asarray(x, dtype=np.float32))
    a = np.ascontiguousarray(np.asarray(cls_attention_map, dtype=np.float32))
    assert x.shape == (B, N, D) and a.shape == (B, NPAT)

    nc = _get_nc()
    pad = np.zeros((XROWS - BPC * N, D), dtype=np.float32)
    in_maps = []
    for c in range(NCORES):
        xs = np.concatenate(
            [x[c * BPC:(c + 1) * BPC].reshape(BPC * N, D), pad], axis=0)
        at = np.concatenate([a[c * BPC:], a[:c * BPC]], axis=0)
        in_maps.append({"x": xs, "attn": at, "dmat": DMAT})

    trace = _bench_out is not None
    res = run_bass_kernel_spmd(nc, in_maps, core_ids=list(range(NCORES)),
                               trace=trace)
    if _bench_out is not None:
        _bench_out["results"] = res

    final = np.empty((B, TOUT, D), dtype=np.float32)
    mask = np.empty((B, TOUT), dtype=bool)
    for c in range(NCORES):
        final[c * BPC:(c + 1) * BPC] = \
            outs_reshape(res.results[c]["out"])
        lev = np.rint(np.asarray(res.results[c]["levels"])).astype(np.int64)
        lev = lev.reshape(BPC)
        out_len = np.where(lev == 1, L1, np.where(lev == 2, L2, L3))
        mask[c * BPC:(c + 1) * BPC, 0] = True
        mask[c * BPC:(c + 1) * BPC, 1:] = \
            np.arange(L1)[None, :] < out_len[:, None]
    return final, mask


def outs_reshape(o):
    return np.asarray(o).reshape(BPC, TOUT, D)


# revision 20
# speedup vs baseline: 1.3956x; 1.3956x over previous
"""Adaptive token pruner (entropy-gated cascaded db4 DWT) on 8 TRN2 NeuronCores.

Strategy (pure data parallel, 32 samples/core):
  - Each core receives its 32-sample shard of x as a flat row tensor
    (32*197 rows + 60 zero pad rows so every K-chunk DMA is 128 rows).
  - Each core receives the FULL cls_attention_map (rotated so its own 32
    samples are rows 0..31) and computes all 256 entropies locally; batch
    quantile thresholds reduce to rank comparisons (q25/q50 interpolation
    lies strictly between order stats 63/64 and 127/128), so
    level masks are m1 = rank>=128, m12 = rank>=64 with
    rank[b] = #{j : s[j] > s[b]}, s[b] = sum_n a*ln(a+1e-9) (= -ln2 * ent).
  - The 1/2/3-level lowpass DWT cascade along the 196 patch tokens is a
    banded matmul with seq as the contraction dim: y_sel = M_b^T @ patch,
    where M_b = D0 + m12*D1 + m1*D2 blends precomputed composite filter
    matrices (D0=C3p, D1=C2p-C3p, D2=W1-C2p); zero padding of shorter
    levels falls out exactly (blended columns are exact zeros).
  - Per sample: 4 fp32r matmuls (2 K-chunks x 2 PSUM banks), one
    PSUM->SBUF copy, grouped DMAs. CLS token rows are copied DRAM->DRAM.
"""

import numpy as np

import concourse.bass as bass
import concourse.mybir as mybir
import concourse.tile as tile
from concourse import bacc
from concourse.bass_utils import run_bass_kernel_spmd
from concourse.masks import make_identity

F32 = mybir.dt.float32
F32R = mybir.dt.float32r
AF = mybir.ActivationFunctionType
ALU = mybir.AluOpType

B, N, D = 256, 197, 768
NCORES = 8
BPC = B // NCORES          # 32 samples per core
NPAT = N - 1               # 196 patch tokens
L1, L2, L3 = 101, 54, 30   # DWT output lengths
TOUT = 1 + L1              # 102 output rows per sample
XROWS = BPC * N + 60       # flat x rows per core + pad so row chunks reach 256
G = 4                      # samples per DMA group

DB4_DEC_LO = np.array([
    -0.010597401784997278, 0.032883011666982945,
    0.030841381835986965, -0.18703481171888114,
    -0.02798376941698385, 0.6308807679295904,
    0.7148465705525415, 0.23037781330885523], dtype=np.float64)


def _build_w(n_in):
    """Banded matrix W (n_in, out_len) with y = W.T @ x equal to one level of
    zero-mode stride-2 db4 lowpass DWT (pytorch_wavelets conv semantics)."""
    h = DB4_DEC_LO[::-1]
    L = h.shape[0]
    out_len = (n_in + L - 1) // 2
    p = 2 * (out_len - 1) - n_in + L
    half = p // 2
    W = np.zeros((n_in, out_len), dtype=np.float64)
    for t in range(out_len):
        for l in range(L):
            n2 = 2 * t + l - half
            if 0 <= n2 < n_in:
                W[n2, t] += h[l]
    return W


def _build_dmat():
    """(128, 3, 2, 101) f32: D[p, m, j, t] = Dm[j*128+p, t], K padded 196->256
    with zeros (pad rows multiply don't-care x rows)."""
    W1 = _build_w(NPAT)              # (196, 101)
    W2 = _build_w(L1)                # (101, 54)
    W3 = _build_w(L2)                # (54, 30)
    C2 = W1 @ W2
    C3 = C2 @ W3
    C2p = np.zeros((NPAT, L1)); C2p[:, :L2] = C2
    C3p = np.zeros((NPAT, L1)); C3p[:, :L3] = C3
    Ds = np.stack([C3p, C2p - C3p, W1 - C2p])      # (3, 196, 101)
    Dk = np.zeros((3, 2 * 128, L1))
    Dk[:, :NPAT, :] = Ds
    return np.ascontiguousarray(
        Dk.reshape(3, 2, 128, L1).transpose(2, 0, 1, 3)).astype(np.float32)


DMAT = _build_dmat()

_NC_CACHE = {}


def _strided(ap, offset, dims):
    c = ap.copy()
    c.ap = c.ap[:0] + [list(d) for d in dims]
    c.offset = offset
    return c


def _build_nc():
    nc = bacc.Bacc("TRN2", target_bir_lowering=False, debug=False,
                   num_devices=NCORES)
    x = nc.dram_tensor("x", [XROWS, D], F32, kind="ExternalInput")
    attn = nc.dram_tensor("attn", [B, NPAT], F32, kind="ExternalInput")
    dmat = nc.dram_tensor("dmat", [128, 3, 2, L1], F32, kind="ExternalInput")
    out = nc.dram_tensor("out", [BPC * TOUT, D], F32, kind="ExternalOutput")
    levels_out = nc.dram_tensor("levels", [1, BPC], F32, kind="ExternalOutput")

    # Constants initialized in a raw-bass preamble (barrier'd) so Tile ops
    # reading them carry no semaphore waits.
    ident_t = nc.alloc_sbuf_tensor("ident_c", [128, 128], F32)
    biast_t = nc.alloc_sbuf_tensor("biast_c", [128, 1], F32)
    ones_t = nc.alloc_sbuf_tensor("ones_c", [1, 128], F32)
    make_identity(nc, ident_t.ap())
    nc.gpsimd.memset(biast_t.ap(), 1e-9)
    nc.gpsimd.memset(ones_t.ap(), 1.0)
    nc.all_engine_barrier()
    ident = ident_t.ap()
    biast = biast_t.ap()
    ones_row = ones_t.ap()

    from contextlib import ExitStack
    with tile.TileContext(nc) as tc, ExitStack() as ctx:
        const = ctx.enter_context(tc.tile_pool(name="const", bufs=1))
        ent = ctx.enter_context(tc.tile_pool(name="ent", bufs=1))
        pps = ctx.enter_context(tc.tile_pool(name="pps", bufs=1, space="PSUM"))
        xpool = ctx.enter_context(tc.tile_pool(name="xin", bufs=3))
        mbpool = ctx.enter_context(tc.tile_pool(name="mb", bufs=8))
        tmpool = ctx.enter_context(tc.tile_pool(name="mbtmp", bufs=2))
        stg = ctx.enter_context(tc.tile_pool(name="stg", bufs=3))
        ppool = ctx.enter_context(tc.tile_pool(name="psum", bufs=3, space="PSUM"))

        # ---- constants ----
        dm = const.tile([128, 3, 2, L1], F32)
        nc.sync.dma_start(dm[:], dmat.ap())

        # ---- entropy -> rank -> level masks prologue ----
        at = ent.tile([128, 2, NPAT], F32)
        nc.sync.dma_start(at[:], attn.ap().rearrange("(j p) d -> p j d", p=128))
        lg = ent.tile([128, 2, NPAT], F32)
        junk0 = ent.tile([128, NPAT], F32)
        junk1 = ent.tile([128, NPAT], F32)
        s_col = ent.tile([128, 2], F32)
        # absorber: pull the attn-DMA wait onto the DVE engine clock so the
        # table-lowered STT ops below carry at most one embedded wait
        absorb = ent.tile([128, 1], F32)
        nc.vector.tensor_copy(absorb[:], at[:, 0, 0:1])
        for j, junk in ((0, junk0), (1, junk1)):
            nc.scalar.activation(lg[:, j, :], at[:, j, :], AF.Ln,
                                 bias=biast[:])
            nc.vector.scalar_tensor_tensor(
                out=junk[:], in0=lg[:, j, :], scalar=1.0, in1=at[:, j, :],
                op0=ALU.mult, op1=ALU.mult, accum_out=s_col[:, j:j + 1])
        # s values of all 256 samples as one broadcast row
        s_row = ent.tile([1, B], F32)
        for j in range(2):
            tp = pps.tile([1, 128], F32, tag="tp")
            nc.tensor.transpose(tp[:], s_col[:, j:j + 1], ident[:])
            nc.vector.tensor_copy(s_row[:, j * 128:(j + 1) * 128], tp[:])
        # broadcast s_row to all 128 partitions via ones (K=1) matmul
        s_bc = ent.tile([128, B], F32)
        bc_ps = pps.tile([128, B], F32, tag="bc")
        nc.tensor.matmul(bc_ps[:], ones_row[:], s_row[:], start=True, stop=True)
        nc.vector.tensor_copy(s_bc[:], bc_ps[:])
        # rank for tile-0 samples (rows 0..31 are this core's own samples)
        cmp = ent.tile([128, B], F32)
        rank = ent.tile([128, 1], F32)
        nc.vector.tensor_scalar(
            out=cmp[:], in0=s_bc[:], scalar1=s_col[:, 0:1], scalar2=None,
            op0=ALU.is_gt, op1=ALU.add, accum_out=rank[:])
        mtile = ent.tile([128, 3], F32)   # cols: m1, m12, level
        nc.vector.tensor_scalar(
            out=mtile[:, 0:1], in0=rank[:], scalar1=128.0, scalar2=None,
            op0=ALU.is_ge)
        nc.vector.tensor_scalar(
            out=mtile[:, 1:2], in0=rank[:], scalar1=64.0, scalar2=None,
            op0=ALU.is_ge)
        # level = 3 - m1 - m12
        nc.vector.scalar_tensor_tensor(
            out=mtile[:, 2:3], in0=mtile[:, 0:1], scalar=-1.0,
            in1=mtile[:, 1:2], op0=ALU.mult, op1=ALU.subtract)
        nc.vector.tensor_scalar(
            out=mtile[:, 2:3], in0=mtile[:, 2:3], scalar1=3.0, scalar2=None,
            op0=ALU.add)
        mrow = ent.tile([1, 2 * BPC], F32)
        lev_row = ent.tile([1, BPC], F32)
        for col, dst in ((0, mrow[:, 0:BPC]), (1, mrow[:, BPC:2 * BPC]),
                         (2, lev_row[:])):
            tpm = pps.tile([1, 128], F32, tag="tp")
            nc.tensor.transpose(tpm[:], mtile[:, col:col + 1], ident[:])
            nc.vector.tensor_copy(dst, tpm[0:1, 0:BPC])
        nc.sync.dma_start(levels_out.ap(), lev_row[:])
        m_bc = ent.tile([128, 2 * BPC], F32)   # [:, i]=m1_i, [:, 32+i]=m12_i
        mb_ps = pps.tile([128, 2 * BPC], F32, tag="bc")
        nc.tensor.matmul(mb_ps[:], ones_row[:], mrow[:], start=True, stop=True)
        nc.vector.tensor_copy(m_bc[:], mb_ps[:])

        # ---- main loop: blended banded matmul per sample ----
        for g in range(BPC // G):
            b0 = g * G
            xt = xpool.tile([128, G, 2, D], F32R)
            for j in range(2):
                nc.gpsimd.dma_start(
                    xt[:, :, j, :],
                    _strided(x.ap(), (N * b0 + 1 + 128 * j) * D,
                             [[D, 128], [N * D, G], [1, D]]))
            st = stg.tile([L1, G, D], F32)
            for s in range(G):
                i = b0 + s
                mb = mbpool.tile([128, 2, L1], F32R)
                for j in range(2):
                    tmpb = tmpool.tile([128, L1], F32, tag="tmpb")
                    nc.vector.scalar_tensor_tensor(
                        out=tmpb[:], in0=dm[:, 1, j, :],
                        scalar=m_bc[:, BPC + i:BPC + i + 1],
                        in1=dm[:, 0, j, :], op0=ALU.mult, op1=ALU.add)
                    nc.vector.scalar_tensor_tensor(
                        out=mb[:, j, :], in0=dm[:, 2, j, :],
                        scalar=m_bc[:, i:i + 1],
                        in1=tmpb[:], op0=ALU.mult, op1=ALU.add)
                ps = ppool.tile([L1, D], F32)
                for j in range(2):
                    for n0, n1 in ((0, 512), (512, D)):
                        nc.tensor.matmul(
                            ps[:, n0:n1],
                            mb[:, j, :],
                            xt[:, s, j, n0:n1],
                            start=(j == 0), stop=(j == 1))
                if s % 2 == 0:
                    nc.vector.tensor_copy(st[:, s, :], ps[:])
                else:
                    nc.scalar.copy(st[:, s, :], ps[:])
            nc.gpsimd.dma_start(
                _strided(out.ap(), (TOUT * b0 + 1) * D,
                         [[D, L1], [TOUT * D, G], [1, D]]),
                st[:])
        # CLS token rows: DRAM -> DRAM strided copy for all 32 samples
        nc.sync.dma_start(
            _strided(out.ap(), 0, [[TOUT * D, BPC], [1, D]]),
            _strided(x.ap(), 0, [[N * D, BPC], [1, D]]))
    nc.compile()
    return nc


def _get_nc():
    if "nc" not in _NC_CACHE:
        _NC_CACHE["nc"] = _build_nc()
    return _NC_CACHE["nc"]


def kernel(x, cls_attention_map, _bench_out=None):
    x = np.ascontiguousarray(np.asarray(x, dtype=np.float32))
    a = np.ascontiguousarray(np.asarray(cls_attention_map, dtype=np.float32))
    assert x.shape == (B, N, D) and a.shape == (B, NPAT)

    nc = _get_nc()
    pad = np.zeros((XROWS - BPC * N, D), dtype=np.float32)
    in_maps = []
    for c in range(NCORES):
        xs = np.concatenate(
            [x[c * BPC:(c + 1) * BPC].reshape(BPC * N, D), pad], axis=0)
        at = np.concatenate([a[c * BPC:], a[:c * BPC]], axis=0)
        in_maps.append({"x": xs, "attn": at, "dmat": DMAT})

    trace = _bench_out is not None
    res = run_bass_kernel_spmd(nc, in_maps, core_ids=list(range(NCORES)),
                               trace=trace)
    if _bench_out is not None:
        _bench_out["results"] = res

    final = np.empty((B, TOUT, D), dtype=np.float32)
    mask = np.empty((B, TOUT), dtype=bool)
    for c in range(NCORES):
        final[c * BPC:(c + 1) * BPC] = \
            outs_reshape(res.results[c]["out"])
        lev = np.rint(np.asarray(res.results[c]["levels"])).astype(np.int64)
        lev = lev.reshape(BPC)
        out_len = np.where(lev == 1, L1, np.where(lev == 2, L2, L3))
        mask[c * BPC:(c + 1) * BPC, 0] = True
        mask[c * BPC:(c + 1) * BPC, 1:] = \
            np.arange(L1)[None, :] < out_len[:, None]
    return final, mask


def outs_reshape(o):
    return np.asarray(o).reshape(BPC, TOUT, D)


# revision 22
# speedup vs baseline: 1.7814x; 1.2765x over previous
"""Adaptive token pruner (entropy-gated cascaded db4 DWT) on 8 TRN2 NeuronCores.

Strategy (pure data parallel, 32 samples/core):
  - Each core receives its 32-sample shard of x as a flat row tensor
    (32*197 rows + 60 zero pad rows so every K-chunk DMA is 128 rows).
  - Each core receives the FULL cls_attention_map (rotated so its own 32
    samples are rows 0..31) and computes all 256 entropies locally; batch
    quantile thresholds reduce to rank comparisons (q25/q50 interpolation
    lies strictly between order stats 63/64 and 127/128), so
    level masks are m1 = rank>=128, m12 = rank>=64 with
    rank[b] = #{j : s[j] > s[b]}, s[b] = sum_n a*ln(a+1e-9) (= -ln2 * ent).
  - The 1/2/3-level lowpass DWT cascade along the 196 patch tokens is a
    banded matmul with seq as the contraction dim: y_sel = M_b^T @ patch,
    where M_b = D0 + m12*D1 + m1*D2 blends precomputed composite filter
    matrices (D0=C3p, D1=C2p-C3p, D2=W1-C2p); zero padding of shorter
    levels falls out exactly (blended columns are exact zeros).
  - Per sample: 4 fp32r matmuls (2 K-chunks x 2 PSUM banks), one
    PSUM->SBUF copy, grouped DMAs. CLS token rows are copied DRAM->DRAM.
"""

import numpy as np

import concourse.bass as bass
import concourse.mybir as mybir
import concourse.tile as tile
from concourse import bacc
from concourse.bass_utils import run_bass_kernel_spmd
from concourse.masks import make_identity

F32 = mybir.dt.float32
F32R = mybir.dt.float32r
AF = mybir.ActivationFunctionType
ALU = mybir.AluOpType

B, N, D = 256, 197, 768
NCORES = 8
BPC = B // NCORES          # 32 samples per core
NPAT = N - 1               # 196 patch tokens
L1, L2, L3 = 101, 54, 30   # DWT output lengths
TOUT = 1 + L1              # 102 output rows per sample
XROWS = BPC * N + 60       # flat x rows per core + pad so row chunks reach 256
G = 4                      # samples per DMA group

DB4_DEC_LO = np.array([
    -0.010597401784997278, 0.032883011666982945,
    0.030841381835986965, -0.18703481171888114,
    -0.02798376941698385, 0.6308807679295904,
    0.7148465705525415, 0.23037781330885523], dtype=np.float64)


def _build_w(n_in):
    """Banded matrix W (n_in, out_len) with y = W.T @ x equal to one level of
    zero-mode stride-2 db4 lowpass DWT (pytorch_wavelets conv semantics)."""
    h = DB4_DEC_LO[::-1]
    L = h.shape[0]
    out_len = (n_in + L - 1) // 2
    p = 2 * (out_len - 1) - n_in + L
    half = p // 2
    W = np.zeros((n_in, out_len), dtype=np.float64)
    for t in range(out_len):
        for l in range(L):
            n2 = 2 * t + l - half
            if 0 <= n2 < n_in:
                W[n2, t] += h[l]
    return W


def _build_dmat():
    """(128, 3, 2, 101) f32: D[p, m, j, t] = Dm[j*128+p, t], K padded 196->256
    with zeros (pad rows multiply don't-care x rows)."""
    W1 = _build_w(NPAT)              # (196, 101)
    W2 = _build_w(L1)                # (101, 54)
    W3 = _build_w(L2)                # (54, 30)
    C2 = W1 @ W2
    C3 = C2 @ W3
    C2p = np.zeros((NPAT, L1)); C2p[:, :L2] = C2
    C3p = np.zeros((NPAT, L1)); C3p[:, :L3] = C3
    Ds = np.stack([C3p, C2p - C3p, W1 - C2p])      # (3, 196, 101)
    Dk = np.zeros((3, 2 * 128, L1))
    Dk[:, :NPAT, :] = Ds
    return np.ascontiguousarray(
        Dk.reshape(3, 2, 128, L1).transpose(2, 0, 1, 3)).astype(np.float32)


DMAT = _build_dmat()

_NC_CACHE = {}


def _strided(ap, offset, dims):
    c = ap.copy()
    c.ap = c.ap[:0] + [list(d) for d in dims]
    c.offset = offset
    return c


def _build_nc():
    nc = bacc.Bacc("TRN2", target_bir_lowering=False, debug=False,
                   num_devices=NCORES)
    x = nc.dram_tensor("x", [XROWS, D], F32, kind="ExternalInput")
    attn = nc.dram_tensor("attn", [B, NPAT], F32, kind="ExternalInput")
    dmat = nc.dram_tensor("dmat", [128, 3, 2, L1], F32, kind="ExternalInput")
    out = nc.dram_tensor("out", [BPC * TOUT, D], F32, kind="ExternalOutput")
    levels_out = nc.dram_tensor("levels", [1, BPC], F32, kind="ExternalOutput")

    # Constants initialized in a raw-bass preamble (barrier'd) so Tile ops
    # reading them carry no semaphore waits.
    ident_t = nc.alloc_sbuf_tensor("ident_c", [128, 128], F32)
    biast_t = nc.alloc_sbuf_tensor("biast_c", [128, 1], F32)
    ones_t = nc.alloc_sbuf_tensor("ones_c", [1, 128], F32)
    make_identity(nc, ident_t.ap())
    nc.gpsimd.memset(biast_t.ap(), 1e-9)
    nc.gpsimd.memset(ones_t.ap(), 1.0)
    nc.all_engine_barrier()
    ident = ident_t.ap()
    biast = biast_t.ap()
    ones_row = ones_t.ap()

    from contextlib import ExitStack
    with tile.TileContext(nc) as tc, ExitStack() as ctx:
        const = ctx.enter_context(tc.tile_pool(name="const", bufs=1))
        ent = ctx.enter_context(tc.tile_pool(name="ent", bufs=1))
        pps = ctx.enter_context(tc.tile_pool(name="pps", bufs=1, space="PSUM"))
        xpool = ctx.enter_context(tc.tile_pool(name="xin", bufs=4))
        mbpool = ctx.enter_context(tc.tile_pool(name="mb", bufs=8))
        tmpool = ctx.enter_context(tc.tile_pool(name="mbtmp", bufs=2))
        stg = ctx.enter_context(tc.tile_pool(name="stg", bufs=4))
        ppool = ctx.enter_context(tc.tile_pool(name="psum", bufs=3, space="PSUM"))

        # ---- constants ----
        dm = const.tile([128, 3, 2, L1], F32)
        nc.sync.dma_start(dm[:], dmat.ap())

        # ---- entropy -> rank -> level masks prologue ----
        at = ent.tile([128, 2, NPAT], F32)
        nc.sync.dma_start(at[:], attn.ap().rearrange("(j p) d -> p j d", p=128))
        lg = ent.tile([128, 2, NPAT], F32)
        junk0 = ent.tile([128, NPAT], F32)
        junk1 = ent.tile([128, NPAT], F32)
        s_col = ent.tile([128, 2], F32)
        # absorber: pull the attn-DMA wait onto the DVE engine clock so the
        # table-lowered STT ops below carry at most one embedded wait
        absorb = ent.tile([128, 1], F32)
        nc.vector.tensor_copy(absorb[:], at[:, 0, 0:1])
        for j, junk in ((0, junk0), (1, junk1)):
            nc.scalar.activation(lg[:, j, :], at[:, j, :], AF.Ln,
                                 bias=biast[:])
            nc.vector.scalar_tensor_tensor(
                out=junk[:], in0=lg[:, j, :], scalar=1.0, in1=at[:, j, :],
                op0=ALU.mult, op1=ALU.mult, accum_out=s_col[:, j:j + 1])
        # s values of all 256 samples as one broadcast row
        s_row = ent.tile([1, B], F32)
        for j in range(2):
            tp = pps.tile([1, 128], F32, tag="tp")
            nc.tensor.transpose(tp[:], s_col[:, j:j + 1], ident[:])
            nc.vector.tensor_copy(s_row[:, j * 128:(j + 1) * 128], tp[:])
        # broadcast s_row to all 128 partitions via ones (K=1) matmul
        s_bc = ent.tile([128, B], F32)
        bc_ps = pps.tile([128, B], F32, tag="bc")
        nc.tensor.matmul(bc_ps[:], ones_row[:], s_row[:], start=True, stop=True)
        nc.vector.tensor_copy(s_bc[:], bc_ps[:])
        # rank for tile-0 samples (rows 0..31 are this core's own samples)
        cmp = ent.tile([128, B], F32)
        rank = ent.tile([128, 1], F32)
        nc.vector.tensor_scalar(
            out=cmp[:], in0=s_bc[:], scalar1=s_col[:, 0:1], scalar2=None,
            op0=ALU.is_gt, op1=ALU.add, accum_out=rank[:])
        mtile = ent.tile([128, 3], F32)   # cols: m1, m12, level
        nc.vector.tensor_scalar(
            out=mtile[:, 0:1], in0=rank[:], scalar1=128.0, scalar2=None,
            op0=ALU.is_ge)
        nc.vector.tensor_scalar(
            out=mtile[:, 1:2], in0=rank[:], scalar1=64.0, scalar2=None,
            op0=ALU.is_ge)
        # level = 3 - m1 - m12
        nc.vector.scalar_tensor_tensor(
            out=mtile[:, 2:3], in0=mtile[:, 0:1], scalar=-1.0,
            in1=mtile[:, 1:2], op0=ALU.mult, op1=ALU.subtract)
        nc.vector.tensor_scalar(
            out=mtile[:, 2:3], in0=mtile[:, 2:3], scalar1=3.0, scalar2=None,
            op0=ALU.add)
        mrow = ent.tile([1, 2 * BPC], F32)
        lev_row = ent.tile([1, BPC], F32)
        for col, dst in ((0, mrow[:, 0:BPC]), (1, mrow[:, BPC:2 * BPC]),
                         (2, lev_row[:])):
            tpm = pps.tile([1, 128], F32, tag="tp")
            nc.tensor.transpose(tpm[:], mtile[:, col:col + 1], ident[:])
            nc.vector.tensor_copy(dst, tpm[0:1, 0:BPC])
        nc.sync.dma_start(levels_out.ap(), lev_row[:])
        m_bc = ent.tile([128, 2 * BPC], F32)   # [:, i]=m1_i, [:, 32+i]=m12_i
        mb_ps = pps.tile([128, 2 * BPC], F32, tag="bc")
        nc.tensor.matmul(mb_ps[:], ones_row[:], mrow[:], start=True, stop=True)
        nc.vector.tensor_copy(m_bc[:], mb_ps[:])

        # ---- main loop: blended banded matmul per sample ----
        for g in range(BPC // G):
            b0 = g * G
            xt = xpool.tile([128, G, 2, D], F32R)
            for j in range(2):
                nc.gpsimd.dma_start(
                    xt[:, :, j, :],
                    _strided(x.ap(), (N * b0 + 1 + 128 * j) * D,
                             [[D, 128], [N * D, G], [1, D]]))
            st = stg.tile([L1, G, D], F32)
            for s in range(G):
                i = b0 + s
                mb = mbpool.tile([128, 2, L1], F32R)
                for j in range(2):
                    tmpb = tmpool.tile([128, L1], F32, tag="tmpb")
                    nc.vector.scalar_tensor_tensor(
                        out=tmpb[:], in0=dm[:, 1, j, :],
                        scalar=m_bc[:, BPC + i:BPC + i + 1],
                        in1=dm[:, 0, j, :], op0=ALU.mult, op1=ALU.add)
                    nc.vector.scalar_tensor_tensor(
                        out=mb[:, j, :], in0=dm[:, 2, j, :],
                        scalar=m_bc[:, i:i + 1],
                        in1=tmpb[:], op0=ALU.mult, op1=ALU.add)
                ps = ppool.tile([L1, D], F32)
                for j in range(2):
                    for n0, n1 in ((0, 512), (512, D)):
                        nc.tensor.matmul(
                            ps[:, n0:n1],
                            mb[:, j, :],
                            xt[:, s, j, n0:n1],
                            start=(j == 0), stop=(j == 1))
                if s % 2 == 0:
                    nc.vector.tensor_copy(st[:, s, :], ps[:])
                else:
                    nc.scalar.copy(st[:, s, :], ps[:])
            for s in range(G):
                nc.gpsimd.dma_start(
                    _strided(out.ap(), (TOUT * (b0 + s) + 1) * D,
                             [[D, L1], [1, D]]),
                    st[:, s, :])
        # CLS token rows: DRAM -> DRAM strided copy for all 32 samples
        nc.sync.dma_start(
            _strided(out.ap(), 0, [[TOUT * D, BPC], [1, D]]),
            _strided(x.ap(), 0, [[N * D, BPC], [1, D]]))
    nc.compile()
    return nc


def _get_nc():
    if "nc" not in _NC_CACHE:
        _NC_CACHE["nc"] = _build_nc()
    return _NC_CACHE["nc"]


def kernel(x, cls_attention_map, _bench_out=None):
    x = np.ascontiguousarray(np.asarray(x, dtype=np.float32))
    a = np.ascontiguousarray(np.asarray(cls_attention_map, dtype=np.float32))
    assert x.shape == (B, N, D) and a.shape == (B, NPAT)

    nc = _get_nc()
    pad = np.zeros((XROWS - BPC * N, D), dtype=np.float32)
    in_maps = []
    for c in range(NCORES):
        xs = np.concatenate(
            [x[c * BPC:(c + 1) * BPC].reshape(BPC * N, D), pad], axis=0)
        at = np.concatenate([a[c * BPC:], a[:c * BPC]], axis=0)
        in_maps.append({"x": xs, "attn": at, "dmat": DMAT})

    trace = _bench_out is not None
    res = run_bass_kernel_spmd(nc, in_maps, core_ids=list(range(NCORES)),
                               trace=trace)
    if _bench_out is not None:
        _bench_out["results"] = res

    final = np.empty((B, TOUT, D), dtype=np.float32)
    mask = np.empty((B, TOUT), dtype=bool)
    for c in range(NCORES):
        final[c * BPC:(c + 1) * BPC] = \
            outs_reshape(res.results[c]["out"])
        lev = np.rint(np.asarray(res.results[c]["levels"])).astype(np.int64)
        lev = lev.reshape(BPC)
        out_len = np.where(lev == 1, L1, np.where(lev == 2, L2, L3))
        mask[c * BPC:(c + 1) * BPC, 0] = True
        mask[c * BPC:(c + 1) * BPC, 1:] = \
            np.arange(L1)[None, :] < out_len[:, None]
    return final, mask


def outs_reshape(o):
    return np.asarray(o).reshape(BPC, TOUT, D)


# revision 24
# speedup vs baseline: 2.6591x; 1.4927x over previous
"""Adaptive token pruner (entropy-gated cascaded db4 DWT) on 8 TRN2 NeuronCores.

Strategy (pure data parallel, 32 samples/core):
  - Each core receives its 32-sample shard of x as a flat row tensor
    (32*197 rows + 60 zero pad rows so every K-chunk DMA is 128 rows).
  - Each core receives the FULL cls_attention_map (rotated so its own 32
    samples are rows 0..31) and computes all 256 entropies locally; batch
    quantile thresholds reduce to rank comparisons (q25/q50 interpolation
    lies strictly between order stats 63/64 and 127/128), so
    level masks are m1 = rank>=128, m12 = rank>=64 with
    rank[b] = #{j : s[j] > s[b]}, s[b] = sum_n a*ln(a+1e-9) (= -ln2 * ent).
  - The 1/2/3-level lowpass DWT cascade along the 196 patch tokens is a
    banded matmul with seq as the contraction dim: y_sel = M_b^T @ patch,
    where M_b = D0 + m12*D1 + m1*D2 blends precomputed composite filter
    matrices (D0=C3p, D1=C2p-C3p, D2=W1-C2p); zero padding of shorter
    levels falls out exactly (blended columns are exact zeros).
  - Per sample: 4 fp32r matmuls (2 K-chunks x 2 PSUM banks), one
    PSUM->SBUF copy, grouped DMAs. CLS token rows are copied DRAM->DRAM.
"""

import numpy as np

import concourse.bass as bass
import concourse.mybir as mybir
import concourse.tile as tile
from concourse import bacc
from concourse.bass_utils import run_bass_kernel_spmd
from concourse.masks import make_identity

F32 = mybir.dt.float32
F32R = mybir.dt.float32r
AF = mybir.ActivationFunctionType
ALU = mybir.AluOpType

B, N, D = 256, 197, 768
NCORES = 8
BPC = B // NCORES          # 32 samples per core
NPAT = N - 1               # 196 patch tokens
L1, L2, L3 = 101, 54, 30   # DWT output lengths
TOUT = 1 + L1              # 102 output rows per sample
XROWS = BPC * N + 60       # flat x rows per core + pad so row chunks reach 256
G = 4                      # samples per DMA group

DB4_DEC_LO = np.array([
    -0.010597401784997278, 0.032883011666982945,
    0.030841381835986965, -0.18703481171888114,
    -0.02798376941698385, 0.6308807679295904,
    0.7148465705525415, 0.23037781330885523], dtype=np.float64)


def _build_w(n_in):
    """Banded matrix W (n_in, out_len) with y = W.T @ x equal to one level of
    zero-mode stride-2 db4 lowpass DWT (pytorch_wavelets conv semantics)."""
    h = DB4_DEC_LO[::-1]
    L = h.shape[0]
    out_len = (n_in + L - 1) // 2
    p = 2 * (out_len - 1) - n_in + L
    half = p // 2
    W = np.zeros((n_in, out_len), dtype=np.float64)
    for t in range(out_len):
        for l in range(L):
            n2 = 2 * t + l - half
            if 0 <= n2 < n_in:
                W[n2, t] += h[l]
    return W


def _build_dmat():
    """(128, 3, 2, 101) f32: D[p, m, j, t] = Dm[j*128+p, t], K padded 196->256
    with zeros (pad rows multiply don't-care x rows)."""
    W1 = _build_w(NPAT)              # (196, 101)
    W2 = _build_w(L1)                # (101, 54)
    W3 = _build_w(L2)                # (54, 30)
    C2 = W1 @ W2
    C3 = C2 @ W3
    C2p = np.zeros((NPAT, L1)); C2p[:, :L2] = C2
    C3p = np.zeros((NPAT, L1)); C3p[:, :L3] = C3
    Ds = np.stack([C3p, C2p - C3p, W1 - C2p])      # (3, 196, 101)
    Dk = np.zeros((3, 2 * 128, L1))
    Dk[:, :NPAT, :] = Ds
    return np.ascontiguousarray(
        Dk.reshape(3, 2, 128, L1).transpose(2, 0, 1, 3)).astype(np.float32)


DMAT = _build_dmat()

_NC_CACHE = {}


def _strided(ap, offset, dims):
    c = ap.copy()
    c.ap = c.ap[:0] + [list(d) for d in dims]
    c.offset = offset
    return c


def _build_nc():
    nc = bacc.Bacc("TRN2", target_bir_lowering=False, debug=False,
                   num_devices=NCORES)
    x = nc.dram_tensor("x", [XROWS, D], F32, kind="ExternalInput")
    attn = nc.dram_tensor("attn", [B, NPAT], F32, kind="ExternalInput")
    dmat = nc.dram_tensor("dmat", [128, 3, 2, L1], F32, kind="ExternalInput")
    out = nc.dram_tensor("out", [BPC * TOUT, D], F32, kind="ExternalOutput")
    levels_out = nc.dram_tensor("levels", [1, BPC], F32, kind="ExternalOutput")

    # Constants initialized in a raw-bass preamble (barrier'd) so Tile ops
    # reading them carry no semaphore waits.
    ident_t = nc.alloc_sbuf_tensor("ident_c", [128, 128], F32)
    biast_t = nc.alloc_sbuf_tensor("biast_c", [128, 1], F32)
    ones_t = nc.alloc_sbuf_tensor("ones_c", [1, 128], F32)
    make_identity(nc, ident_t.ap())
    nc.gpsimd.memset(biast_t.ap(), 1e-9)
    nc.gpsimd.memset(ones_t.ap(), 1.0)
    nc.all_engine_barrier()
    ident = ident_t.ap()
    biast = biast_t.ap()
    ones_row = ones_t.ap()

    from contextlib import ExitStack
    with tile.TileContext(nc) as tc, ExitStack() as ctx:
        const = ctx.enter_context(tc.tile_pool(name="const", bufs=1))
        ent = ctx.enter_context(tc.tile_pool(name="ent", bufs=1))
        pps = ctx.enter_context(tc.tile_pool(name="pps", bufs=1, space="PSUM"))
        xpool = ctx.enter_context(tc.tile_pool(name="xin", bufs=4))
        mbpool = ctx.enter_context(tc.tile_pool(name="mb", bufs=8))
        tmpool = ctx.enter_context(tc.tile_pool(name="mbtmp", bufs=2))
        stg = ctx.enter_context(tc.tile_pool(name="stg", bufs=4))
        ppool = ctx.enter_context(tc.tile_pool(name="psum", bufs=3, space="PSUM"))

        # ---- constants ----
        dm = const.tile([128, 3, 2, L1], F32)
        nc.sync.dma_start(dm[:], dmat.ap())

        # ---- entropy -> rank -> level masks prologue ----
        at = ent.tile([128, 2, NPAT], F32)
        nc.sync.dma_start(at[:], attn.ap().rearrange("(j p) d -> p j d", p=128))
        lg = ent.tile([128, 2, NPAT], F32)
        junk0 = ent.tile([128, NPAT], F32)
        junk1 = ent.tile([128, NPAT], F32)
        s_col = ent.tile([128, 2], F32)
        # absorber: pull the attn-DMA wait onto the DVE engine clock so the
        # table-lowered STT ops below carry at most one embedded wait
        absorb = ent.tile([128, 1], F32)
        nc.vector.tensor_copy(absorb[:], at[:, 0, 0:1])
        for j, junk in ((0, junk0), (1, junk1)):
            nc.scalar.activation(lg[:, j, :], at[:, j, :], AF.Ln,
                                 bias=biast[:])
            nc.vector.scalar_tensor_tensor(
                out=junk[:], in0=lg[:, j, :], scalar=1.0, in1=at[:, j, :],
                op0=ALU.mult, op1=ALU.mult, accum_out=s_col[:, j:j + 1])
        # s values of all 256 samples as one broadcast row
        s_row = ent.tile([1, B], F32)
        for j in range(2):
            tp = pps.tile([1, 128], F32, tag="tp")
            nc.tensor.transpose(tp[:], s_col[:, j:j + 1], ident[:])
            nc.vector.tensor_copy(s_row[:, j * 128:(j + 1) * 128], tp[:])
        # broadcast s_row to all 128 partitions via ones (K=1) matmul
        s_bc = ent.tile([128, B], F32)
        bc_ps = pps.tile([128, B], F32, tag="bc")
        nc.tensor.matmul(bc_ps[:], ones_row[:], s_row[:], start=True, stop=True)
        nc.vector.tensor_copy(s_bc[:], bc_ps[:])
        # rank for tile-0 samples (rows 0..31 are this core's own samples)
        cmp = ent.tile([128, B], F32)
        rank = ent.tile([128, 1], F32)
        nc.vector.tensor_scalar(
            out=cmp[:], in0=s_bc[:], scalar1=s_col[:, 0:1], scalar2=None,
            op0=ALU.is_gt, op1=ALU.add, accum_out=rank[:])
        mtile = ent.tile([128, 3], F32)   # cols: m1, m12, level
        nc.vector.tensor_scalar(
            out=mtile[:, 0:1], in0=rank[:], scalar1=128.0, scalar2=None,
            op0=ALU.is_ge)
        nc.vector.tensor_scalar(
            out=mtile[:, 1:2], in0=rank[:], scalar1=64.0, scalar2=None,
            op0=ALU.is_ge)
        # level = 3 - m1 - m12
        nc.vector.scalar_tensor_tensor(
            out=mtile[:, 2:3], in0=mtile[:, 0:1], scalar=-1.0,
            in1=mtile[:, 1:2], op0=ALU.mult, op1=ALU.subtract)
        nc.vector.tensor_scalar(
            out=mtile[:, 2:3], in0=mtile[:, 2:3], scalar1=3.0, scalar2=None,
            op0=ALU.add)
        mrow = ent.tile([1, 2 * BPC], F32)
        lev_row = ent.tile([1, BPC], F32)
        for col, dst in ((0, mrow[:, 0:BPC]), (1, mrow[:, BPC:2 * BPC]),
                         (2, lev_row[:])):
            tpm = pps.tile([1, 128], F32, tag="tp")
            nc.tensor.transpose(tpm[:], mtile[:, col:col + 1], ident[:])
            nc.vector.tensor_copy(dst, tpm[0:1, 0:BPC])
        nc.sync.dma_start(levels_out.ap(), lev_row[:])
        m_bc = ent.tile([128, 2 * BPC], F32)   # [:, i]=m1_i, [:, 32+i]=m12_i
        mb_ps = pps.tile([128, 2 * BPC], F32, tag="bc")
        nc.tensor.matmul(mb_ps[:], ones_row[:], mrow[:], start=True, stop=True)
        nc.vector.tensor_copy(m_bc[:], mb_ps[:])

        # ---- main loop: blended banded matmul per sample ----
        NG = BPC // G
        LOOKAHEAD = 3

        def load_group(g):
            b0 = g * G
            xt = xpool.tile([128, G, 2, D], F32R, tag="xt")
            for j in range(2):
                nc.gpsimd.dma_start(
                    xt[:, :, j, :],
                    _strided(x.ap(), (N * b0 + 1 + 128 * j) * D,
                             [[D, 128], [N * D, G], [1, D]]))
            return xt

        xts = {g: load_group(g) for g in range(LOOKAHEAD)}
        for g in range(NG):
            b0 = g * G
            xt = xts.pop(g)
            st = stg.tile([L1, G, D], F32)
            for s in range(G):
                i = b0 + s
                mb = mbpool.tile([128, 2, L1], F32R)
                for j in range(2):
                    tmpb = tmpool.tile([128, L1], F32, tag="tmpb")
                    nc.vector.scalar_tensor_tensor(
                        out=tmpb[:], in0=dm[:, 1, j, :],
                        scalar=m_bc[:, BPC + i:BPC + i + 1],
                        in1=dm[:, 0, j, :], op0=ALU.mult, op1=ALU.add)
                    nc.vector.scalar_tensor_tensor(
                        out=mb[:, j, :], in0=dm[:, 2, j, :],
                        scalar=m_bc[:, i:i + 1],
                        in1=tmpb[:], op0=ALU.mult, op1=ALU.add)
                ps = ppool.tile([L1, D], F32)
                for j in range(2):
                    for n0, n1 in ((0, 512), (512, D)):
                        nc.tensor.matmul(
                            ps[:, n0:n1],
                            mb[:, j, :],
                            xt[:, s, j, n0:n1],
                            start=(j == 0), stop=(j == 1))
                if s % 2 == 0:
                    nc.vector.tensor_copy(st[:, s, :], ps[:])
                else:
                    nc.scalar.copy(st[:, s, :], ps[:])
                nc.gpsimd.dma_start(
                    _strided(out.ap(), (TOUT * (b0 + s) + 1) * D,
                             [[D, L1], [1, D]]),
                    st[:, s, :])
            if g + LOOKAHEAD < NG:
                xts[g + LOOKAHEAD] = load_group(g + LOOKAHEAD)
        # CLS token rows: DRAM -> DRAM strided copy for all 32 samples
        nc.sync.dma_start(
            _strided(out.ap(), 0, [[TOUT * D, BPC], [1, D]]),
            _strided(x.ap(), 0, [[N * D, BPC], [1, D]]))
    nc.compile()
    return nc


def _get_nc():
    if "nc" not in _NC_CACHE:
        _NC_CACHE["nc"] = _build_nc()
    return _NC_CACHE["nc"]


def kernel(x, cls_attention_map, _bench_out=None):
    x = np.ascontiguousarray(np.asarray(x, dtype=np.float32))
    a = np.ascontiguousarray(np.asarray(cls_attention_map, dtype=np.float32))
    assert x.shape == (B, N, D) and a.shape == (B, NPAT)

    nc = _get_nc()
    pad = np.zeros((XROWS - BPC * N, D), dtype=np.float32)
    in_maps = []
    for c in range(NCORES):
        xs = np.concatenate(
            [x[c * BPC:(c + 1) * BPC].reshape(BPC * N, D), pad], axis=0)
        at = np.concatenate([a[c * BPC:], a[:c * BPC]], axis=0)
        in_maps.append({"x": xs, "attn": at, "dmat": DMAT})

    trace = _bench_out is not None
    res = run_bass_kernel_spmd(nc, in_maps, core_ids=list(range(NCORES)),
                               trace=trace)
    if _bench_out is not None:
        _bench_out["results"] = res

    final = np.empty((B, TOUT, D), dtype=np.float32)
    mask = np.empty((B, TOUT), dtype=bool)
    for c in range(NCORES):
        final[c * BPC:(c + 1) * BPC] = \
            outs_reshape(res.results[c]["out"])
        lev = np.rint(np.asarray(res.results[c]["levels"])).astype(np.int64)
        lev = lev.reshape(BPC)
        out_len = np.where(lev == 1, L1, np.where(lev == 2, L2, L3))
        mask[c * BPC:(c + 1) * BPC, 0] = True
        mask[c * BPC:(c + 1) * BPC, 1:] = \
            np.arange(L1)[None, :] < out_len[:, None]
    return final, mask


def outs_reshape(o):
    return np.asarray(o).reshape(BPC, TOUT, D)
